# revision 25
# baseline (speedup 1.0000x reference)
"""Trainium2 Bass kernel for the AttentionLayer problem.

Computation (per batch b):
    keys' = keys + sinenc(text_pos, w=1.385);  query' = query + sinenc(frame_pos, w=1.0)
    q = query' @ Wq + bq ; k = keys' @ Wk + bk ; v = values @ Wv + bv
    scores = q @ k^T ; masked softmax over keys -> attn  (output 1)
    out = (attn @ v) * sqrt(1/512) @ Wo + bo             (output 2)

Device strategy: data-parallel over B=64 across 8 cores (8 batches/core).

Weight folding (the big lever): q and k appear ONLY in scores, and v/x
only flow into out, so adjacent projections fold into single GEMMs:
    scores = q'' @ (Wq Wk^T) @ k''^T   with q'' = query + pe_q, k'' = keys + pe_k
      -> device computes  k~ = Wqk @ k''^T  (k-proj shaped),
         then scores = k~^T-contracted with q'' directly: the entire
         q-projection disappears.  Per-query additive terms
         (q_row.bk, bq.bk) are softmax-invariant and dropped; the
         per-key term bq.(k_row+bk) is a host-computed scalar folded
         into the exp bias alongside the mask.
    out = attn @ (values @ (Wv Wo * sqrt(1/TK))) + (bv@Wo*scale + bo)
      -> Wvo := Wv@Wo*scale folds the v-projection and out-projection
         into one; attn rows sum to 1 so bv folds into the constant
         bias (as before).
PE work per batch drops 163->98 streamed 512-col matmul units (-40%).
Numerics (host sim vs f64 reference): out 2.5e-3 / attn 3.6e-3
rel-to-max -- slightly BETTER than the unfolded fp16 scheme (fewer
quantization points), vs the 2e-2 gate.

Transposed layout ([feature, time]) throughout; no on-device transposes:
    k~T[eq,k] = WqkT^T @ keysT + Kpe    (Kpe = ((pe_k@Wk+bk)@Wq^T)^T, host)
    v~T[k,c]  = valuesT^T-slices @ Wvo
    scoresT[k,q] = k~T^T @ q''T          exp via ACT(Exp, bias=mask+ck)
    denom[q] = ones^T @ (tree-summed exp)    attnT = expT * (1/denom)
    outT[c,q] = v~^T @ attnT + bo'       (single GEMM, bias in the drain)

Dtype: all matmuls fp16 (e5m10), f32 PSUM; exp/denominator in f32
(scores reach +62, exp ~1e27).  fp8 was measured (host sim) at 0.4-0.9
rel err on either path -- unusable.  attn and out are DMA'd as fp16 and
widened on host.

Pipelining: per-batch phases issue as
    v~proj(b) | k~proj(b) | sums(b-1) | scores+exp(b) | x+out(b-1)
so the exp->tree-sum chain of b-1 is covered by ~7us of projection
matmuls and the normalized attn tiles are ready before x(b-1) needs
them.  Warmup matmuls run on a memset tile (no DMA dependency) so the
PE p-state ramp is paid during the initial input DMAs.  The final
batch's out DMAs fan out across all four engine queues to shorten the
drain tail.
"""

import math
import os

import numpy as np

import concourse.tile as tile
from concourse import bacc, mybir
from concourse.bass_utils import run_bass_kernel_spmd

dt = mybir.dt
F32 = dt.float32
F32R = dt.float32r
FP16 = dt.float16
AF = mybir.ActivationFunctionType

B, TQ, TK = 64, 1024, 512
CH = 512          # conv_channels == embed_dim == att_hid
N_CORES = 8
BPC = B // N_CORES  # batches per core
KEY_POS_RATE = 1.385
QUERY_POS_RATE = 1.0
OUT_SCALE = math.sqrt(1.0 / TK)
MASK_NEG = -1.0e30
N_WARMUP = 64

_LAST_EXEC_NS = None
_LAST_RES = None


def _sin_pos_enc(pos, w, d):
    """Reference-exact sinusoidal table for one position vector. [T, d] f32."""
    pos = pos.astype(np.float32)
    i = np.arange(d)
    inv_freq = np.power(np.float32(10000.0), -(2.0 * (i // 2)).astype(np.float32) / d)
    ang = (pos * np.float32(w))[:, None] * inv_freq[None, :]
    pe = np.where(i[None, :] % 2 == 0, np.sin(ang), np.cos(ang)).astype(np.float32)
    pe[pos == 0] = 0.0
    return pe


def _build_program(n_batch, keff=TK):
    nc = bacc.Bacc("TRN2", target_bir_lowering=False, debug=False, num_devices=1)

    qT_d = nc.dram_tensor("qT", [n_batch, CH, TQ], FP16, kind="ExternalInput")
    kT_d = nc.dram_tensor("kT", [n_batch, CH, TK], FP16, kind="ExternalInput")
    vT_d = nc.dram_tensor("vT", [n_batch, CH, TK], FP16, kind="ExternalInput")
    kpe_d = nc.dram_tensor("kpe", [CH, TK], FP16, kind="ExternalInput")
    wqk_d = nc.dram_tensor("wqk", [CH, CH], FP16, kind="ExternalInput")
    wvo_d = nc.dram_tensor("wvo", [CH, CH], FP16, kind="ExternalInput")
    bo_d = nc.dram_tensor("bo", [128, CH], F32, kind="ExternalInput")
    mb_d = nc.dram_tensor("mb", [n_batch, TK], F32, kind="ExternalInput")
    ones_d = nc.dram_tensor("ones", [128, 128], F32R, kind="ExternalInput")

    attn_d = nc.dram_tensor("attnT", [n_batch, TK, TQ], FP16, kind="ExternalOutput")
    out_d = nc.dram_tensor("outT", [n_batch, CH, TQ], FP16, kind="ExternalOutput")

    NQ2 = TQ // 512                   # 2 query chunks of 512
    NCT = CH // 128                   # 4 feature tiles
    NKT = TK // 128                   # 4 key tiles
    s512 = lambda c: slice(c * 512, (c + 1) * 512)
    s128 = lambda t: slice(t * 128, (t + 1) * 128)

    with tile.TileContext(nc) as tc:
        with (
            tc.tile_pool(name="wpool", bufs=1) as wpool,
            tc.tile_pool(name="qin", bufs=3) as p_qin,
            tc.tile_pool(name="kin", bufs=3) as p_kin,
            tc.tile_pool(name="vin", bufs=3) as p_vin,
            tc.tile_pool(name="kt", bufs=5) as p_kt,
            tc.tile_pool(name="vt", bufs=8) as p_vt,
            tc.tile_pool(name="vt7", bufs=4) as p_vt7,
            tc.tile_pool(name="exp", bufs=5) as p_exp,
            tc.tile_pool(name="esum", bufs=1) as p_esum,
            tc.tile_pool(name="rec", bufs=2) as p_rec,
            tc.tile_pool(name="attnb", bufs=3) as p_attnb,
            tc.tile_pool(name="outt", bufs=2) as p_out,
            tc.tile_pool(name="ps", bufs=8, space="PSUM") as p_ps,
        ):
            ps_one = lambda nm: p_ps.tile([128, 512], F32, name=nm, tag="ps")

            # ---- warmup: lift the PE p-state during initial DMAs.
            # memset (ScalarE) instead of a DMA'd table so the first
            # warmup matmul issues ~us after kernel start, before the
            # DMA rings have even initialized. ----
            warm = wpool.tile([128, 128], FP16, name="warm")
            nc.vector.memset(warm[:], 1.0)
            for i in range(N_WARMUP):
                ps = ps_one(f"warm{i}")
                nc.tensor.matmul(
                    ps[:, :128], warm[:], warm[:],
                    start=True, stop=True,
                )

            # ---- resident weights/constants ----
            # ONE coalesced descriptor per tensor: each dma_start costs
            # ~0.7-1us of sequencer issue time (DIRECT2D), and the serial
            # issue stream -- not DMA bandwidth -- paces the lead-in.
            # sync queue carries the critical path: wvo + batch-0 inputs.
            def load_w(name, dram, dtype, engine=nc.sync):
                t = wpool.tile([128, NCT, CH], dtype, name=name)
                engine.dma_start(
                    t[:], dram.ap().rearrange("(c p) e -> p c e", p=128)
                )
                return [t[:, ct, :] for ct in range(NCT)]

            wqk_w = load_w("wqk", wqk_d, FP16)

            # one-time tables on the gpsimd queue (sync stays free for
            # batch-0 inputs).  bo is pre-broadcast to [128, CH] on host
            # and folded into v~ at the vproj drain (attn rows sum to 1,
            # so adding bo to every v~ row is exact) -- the x drains
            # become plain copies.
            bo_sb = wpool.tile([128, CH], F32, name="bob")
            nc.gpsimd.dma_start(bo_sb[:], bo_d.ap())
            # exp bias (mask + per-key score bias) for all batches in one
            # transfer: col b*NKT+j holds mb[b, j*128:(j+1)*128]
            mb_all = wpool.tile([128, n_batch * NKT], F32, name="mball")
            nc.gpsimd.dma_start(
                mb_all[:], mb_d.ap().rearrange("b (j p) -> p (b j)", p=128)
            )
            ones_sb = wpool.tile([128, 128], F32R, name="ones")
            nc.gpsimd.dma_start(ones_sb[:], ones_d.ap())
            kpe_sb = []  # loaded lazily after batch-0 inputs (first use
            # is the kproj drain, ~7us in; keeps the lead-in DMA window
            # free for the critical first-batch tiles)

            state = {}

            def load_in(pool, dram, b, width, name, engine):
                t = pool.tile([128, NCT, width], FP16, name=name, tag=name[:3])
                engine.dma_start(
                    t[:], dram.ap()[b].rearrange("(c p) x -> p c x", p=128)
                )
                return [t[:, ct, :] for ct in range(NCT)]

            # ALL loads on the sync DGE queue, issued in first-use
            # order: a single queue delivers strictly FIFO, so the
            # first-needed tensor completes after exactly its own bytes.
            # (Measured: splitting across two queues makes the rings
            # interleave both streams and the first tensor arrives at
            # the fair-share point -- 7us later.)
            def load_qin(b):
                return load_in(p_qin, qT_d, b, TQ, f"qin{b}", nc.sync)

            def load_kin(b):
                return load_in(p_kin, kT_d, b, TK, f"kin{b}", nc.sync)

            def load_vin(b):
                return load_in(p_vin, vT_d, b, TK, f"vin{b}", nc.sync)

            def kv_part(b, last=False):
                # ---- inputs, in first-use order (kproj leads: its
                # weights+inputs are the first 1MB on the wire, so the
                # PE gets ~3.5us of kproj cover while vin streams) ----
                kin = load_kin(b)
                if state.get("wvo") is None:
                    state["wvo"] = load_w("wvo", wvo_d, FP16)
                vin = load_vin(b)
                qin = load_qin(b)
                if not kpe_sb:
                    t = wpool.tile([128, NCT, TK], FP16, name="kpe")
                    nc.gpsimd.dma_start(
                        t[:], kpe_d.ap().rearrange("(c p) k -> p c k", p=128)
                    )
                    kpe_sb.extend(t[:, ct, :] for ct in range(NCT))
                wvo = state["wvo"]

                # ---- k~ = Wqk @ keys''T (+ table): [eq, k] tiles ----
                # masked trailing keys (cols keff:TK) never reach the
                # output: their exp is forced to exact 0 by the -1e30
                # bias, so k~ for them may hold stale data -- stream only
                # the first keff columns
                kt = []
                for ht in range(NCT):
                    ps = ps_one(f"psk{b}_{ht}")
                    for ct in range(NCT):
                        nc.tensor.matmul(
                            ps[:, :keff], wqk_w[ct][:, s128(ht)],
                            kin[ct][:, :keff],
                            start=(ct == 0), stop=(ct == NCT - 1),
                        )
                    t = p_kt.tile([128, TK], FP16, name=f"kt{b}_{ht}", tag="kt")
                    if keff < TK and b < 2:
                        # first touch of each of the 5 pool buffers: the
                        # stale-col region must be finite (NaN would
                        # poison exp despite the -1e30 bias)
                        nc.vector.memset(t[:, keff:], 0.0)
                    nc.vector.tensor_add(
                        t[:, :keff], ps[:, :keff], kpe_sb[ht][:, :keff]
                    )
                    kt.append(t)

                if b == 0:
                    # fill the wvo/vin DMA window (~4us) with dummy
                    # matmuls: keeps the p-state ramped so vproj starts
                    # at full clock instead of paying the 3us mid-clock
                    # ramp after an idle gap
                    for i in range(32):
                        ps = ps_one(f"fill{i}")
                        nc.tensor.matmul(
                            ps[:, :128], warm[:], warm[:],
                            start=True, stop=True,
                        )

                # ---- v~ = values @ (Wv Wo * scale) + bo: [k, c] tiles.
                # The last batch's v~ is kept f32r so its x-phase can
                # consume the unnormalized exp tiles directly. ----
                vt = []
                for ktile in range(NKT):
                    ps = ps_one(f"psv{b}_{ktile}")
                    for ct in range(NCT):
                        nc.tensor.matmul(
                            ps[:], vin[ct][:, s128(ktile)], wvo[ct][:],
                            start=(ct == 0), stop=(ct == NCT - 1),
                        )
                    if last:
                        t = p_vt7.tile([128, CH], F32R, name=f"vt7_{ktile}",
                                       tag="vt7")
                    else:
                        t = p_vt.tile([128, CH], FP16, name=f"vt{b}_{ktile}",
                                      tag="vt")
                    nc.vector.tensor_add(t[:], ps[:], bo_sb[:])
                    vt.append(t)
                return qin, kt, vt

            def scores_exp(b, qt, kt):
                # ---- scores + exp (mask & key-bias folded into bias) ----
                expt = []
                for ktile in range(NKT):
                    ps = [ps_one(f"pss{b}_{ktile}_{c}") for c in range(NQ2)]
                    for ht in range(NCT):
                        for c in range(NQ2):
                            nc.tensor.matmul(
                                ps[c][:], kt[ht][:, s128(ktile)],
                                qt[ht][:, s512(c)],
                                start=(ht == 0), stop=(ht == NCT - 1),
                            )
                    t = p_exp.tile([128, TQ], F32R, name=f"exp{b}_{ktile}", tag="exp")
                    for c in range(NQ2):
                        nc.scalar.activation(
                            t[:, s512(c)], ps[c][:], AF.Exp,
                            bias=mb_all[:, b * NKT + ktile:b * NKT + ktile + 1],
                        )
                    expt.append(t)
                return expt

            def sums_recip(b, expt):
                # tree-sum the 4 exp tiles on GpSimd, then 2 ones-matmuls
                s01 = p_esum.tile([128, TQ], F32R, name=f"s01_{b}", tag="esA")
                s23 = p_esum.tile([128, TQ], F32R, name=f"s23_{b}", tag="esB")
                s = p_esum.tile([128, TQ], F32R, name=f"s_{b}", tag="esC")
                nc.gpsimd.tensor_add(s01[:], expt[0][:], expt[1][:])
                nc.gpsimd.tensor_add(s23[:], expt[2][:], expt[3][:])
                nc.gpsimd.tensor_add(s[:], s01[:], s23[:])
                rec = p_rec.tile([128, TQ], F32, name=f"rec{b}", tag="rec")
                for c in range(NQ2):
                    ps = ps_one(f"pssum{b}_{c}")
                    nc.tensor.matmul(
                        ps[:], ones_sb[:], s[:, s512(c)],
                        start=True, stop=True,
                    )
                    nc.vector.reciprocal_approx_fast(rec[:, s512(c)], ps[:])
                return rec

            def attn_norm(b, expt, rec):
                # one big [128, NKT*TQ] tile, one output descriptor
                t = p_attnb.tile(
                    [128, NKT, TQ], FP16, name=f"ab{b}", tag="attnb"
                )
                attn = [t[:, ktile, :] for ktile in range(NKT)]
                for ktile in range(NKT):
                    for c in range(NQ2):
                        nc.vector.tensor_mul(
                            attn[ktile][:, s512(c)], expt[ktile][:, s512(c)],
                            rec[:, s512(c)],
                        )
                nc.gpsimd.dma_start(
                    attn_d.ap()[b].rearrange("(j p) q -> p j q", p=128), t[:]
                )
                return attn

            def x_out(b, vt, attn):
                # out = attn @ (v~ + bo) in one GEMM; drain is a plain
                # copy into one big tile, one output descriptor
                t = p_out.tile([128, NCT, TQ], FP16, name=f"ot{b}", tag="outt")
                for ct in range(NCT):
                    ps = [ps_one(f"psx{b}_{ct}_{c}") for c in range(NQ2)]
                    for ktile in range(NKT):
                        for c in range(NQ2):
                            nc.tensor.matmul(
                                ps[c][:], vt[ktile][:, s128(ct)],
                                attn[ktile][:, s512(c)],
                                start=(ktile == 0), stop=(ktile == NKT - 1),
                            )
                    for c in range(NQ2):
                        # alternate psum-drain engines so the 8 drains don't
                        # serialize on one engine
                        dst = t[:, ct, s512(c)]
                        if ct % 2 == 0:
                            nc.scalar.copy(dst, ps[c][:])
                        else:
                            nc.vector.tensor_copy(dst, ps[c][:])
                nc.gpsimd.dma_start(
                    out_d.ap()[b].rearrange("(c p) q -> p c q", p=128), t[:]
                )

            def x_out_last(b, vt, expt, rec):
                # Final batch: consume UNNORMALIZED exp tiles (f32r) and
                # fold 1/denom into the drain multiply, so the x matmuls
                # start right after the sums matmuls with no wait on
                # attn-norm.  rec columns are partition-replicated, so the
                # drain is a single tensor-tensor multiply per chunk.
                # attn-norm muls interleave here, split Vector/GpSimd, so
                # no single engine serializes the endgame; attn DMAs flush
                # per-tile during the x matmuls.
                at = p_attnb.tile(
                    [128, NKT, TQ], FP16, name=f"ab{b}", tag="attnb"
                )
                dma_engines = [nc.gpsimd, nc.scalar, nc.sync]
                for ct in range(NCT):
                    ps = [ps_one(f"psx{b}_{ct}_{c}") for c in range(NQ2)]
                    for ktile in range(NKT):
                        for c in range(NQ2):
                            nc.tensor.matmul(
                                ps[c][:], vt[ktile][:, s128(ct)],
                                expt[ktile][:, s512(c)],
                                start=(ktile == 0), stop=(ktile == NKT - 1),
                            )
                    # attn tile `ct`: vector does chunk 0, gpsimd chunk 1
                    asl = at[:, ct, :]
                    nc.vector.tensor_mul(
                        asl[:, s512(0)], expt[ct][:, s512(0)], rec[:, s512(0)]
                    )
                    nc.gpsimd.tensor_mul(
                        asl[:, s512(1)], expt[ct][:, s512(1)], rec[:, s512(1)]
                    )
                    nc.gpsimd.dma_start(attn_d.ap()[b, s128(ct), :], asl)
                    t = p_out.tile([128, TQ], FP16, name=f"ot7_{ct}", tag="out7")
                    for c in range(NQ2):
                        nc.vector.tensor_mul(
                            t[:, s512(c)], ps[c][:], rec[:, s512(c)]
                        )
                        # chunked final store across three engine queues:
                        # start draining sooner, flush faster
                        eng = dma_engines[(ct * NQ2 + c) % 3]
                        eng.dma_start(
                            out_d.ap()[b, s128(ct), s512(c)], t[:, s512(c)]
                        )

            # 2-deep pipeline: per iteration
            #   v~proj(b) k~proj(b) | sums(b-1) | scores+exp(b) | x+out(b-1)
            # exp(b-1)'s tree-sum is covered by both projections (~7us);
            # recip+attn-norm(b-1) by scores(b).
            prev = None  # (expt, vt) of previous batch
            for b in range(n_batch):
                qin, kt, vt = kv_part(b, last=(b == n_batch - 1))
                if prev is not None:
                    rec = sums_recip(b - 1, prev[0])
                    attn = attn_norm(b - 1, prev[0], rec)
                expt = scores_exp(b, qin, kt)
                if prev is not None:
                    x_out(b - 1, prev[1], attn)
                prev = (expt, vt)
            b = n_batch - 1
            rec = sums_recip(b, prev[0])
            x_out_last(b, prev[1], prev[0], rec)
    nc.compile()
    return nc


def _host_prep(inputs):
    query = np.asarray(inputs["query"], dtype=np.float32)
    keys = np.asarray(inputs["keys"], dtype=np.float32)
    values = np.asarray(inputs["values"], dtype=np.float32)
    tpos = np.asarray(inputs["text_positions"])
    fpos = np.asarray(inputs["frame_positions"])
    mask = np.asarray(inputs["mask"])
    Wq = np.asarray(inputs["Wq"], dtype=np.float32)
    Wk = np.asarray(inputs["Wk"], dtype=np.float32)
    Wv = np.asarray(inputs["Wv"], dtype=np.float32)
    Wo = np.asarray(inputs["Wo"], dtype=np.float32)
    bq = np.asarray(inputs["bq"], dtype=np.float32)
    bk = np.asarray(inputs["bk"], dtype=np.float32)
    bv = np.asarray(inputs["bv"], dtype=np.float32)
    bo = np.asarray(inputs["bo"], dtype=np.float32)

    fshared = bool(np.all(fpos == fpos[0:1]))
    tshared = bool(np.all(tpos == tpos[0:1]))

    # q'' = query + pe_q, quantized once (no q-projection on device)
    if fshared:
        peq = _sin_pos_enc(fpos[0], QUERY_POS_RATE, CH)       # [TQ, CH]
        qT = (query + peq[None]).transpose(0, 2, 1)
    else:
        peq = np.stack([_sin_pos_enc(p, QUERY_POS_RATE, CH) for p in fpos])
        qT = (query + peq).transpose(0, 2, 1)

    # k'' = keys + pe_k; pe routed through the f32 table when shared
    wkq = Wk @ Wq.T                                            # [ek, eq]
    if tshared:
        pek = _sin_pos_enc(tpos[0], KEY_POS_RATE, CH)          # [TK, CH]
        kT = keys.transpose(0, 2, 1)
        kpe = np.ascontiguousarray(((pek @ Wk + bk) @ Wq.T).T)  # [eq, TK]
        keys_full = keys + pek[None]
    else:
        pek = np.stack([_sin_pos_enc(p, KEY_POS_RATE, CH) for p in tpos])
        kT = (keys + pek).transpose(0, 2, 1)
        kpe = np.ascontiguousarray(
            np.broadcast_to((bk @ Wq.T)[:, None], (CH, TK))
        ).astype(np.float32)
        keys_full = keys + pek

    qT = np.ascontiguousarray(qT).astype(np.float16)
    kT = np.ascontiguousarray(kT).astype(np.float16)
    vT = np.ascontiguousarray(values.transpose(0, 2, 1)).astype(np.float16)

    # exp bias: mask plus the per-key bq.(k_row+bk) term the q-side
    # folding moved out of the scores GEMM (zero when bq == 0)
    ck = keys_full @ (Wk @ bq) + np.float32(bk @ bq)           # [B, TK]
    mb = np.where(mask, np.float32(MASK_NEG), np.float32(0.0)) + ck
    mb = mb.astype(np.float32)
    ones = np.ones((128, 128), dtype=np.float32)

    # masked-tail fast path: if every batch masks exactly the last 64
    # keys, the k-projection can skip those columns (their exp is forced
    # to exact zero by the bias regardless of the k~ values there)
    keff = TK
    if bool(np.array_equal(
        mask, np.broadcast_to(np.arange(TK) >= TK - 64, mask.shape)
    )):
        keff = TK - 64

    bo2 = (np.float32(OUT_SCALE) * (bv @ Wo) + bo).astype(np.float32)
    shared = {
        "wqk": wkq.astype(np.float16),
        "wvo": (Wv @ Wo * np.float32(OUT_SCALE)).astype(np.float16),
        "kpe": kpe.astype(np.float16),
        "bo": np.ascontiguousarray(np.broadcast_to(bo2[None, :], (128, CH))),
        "ones": ones,
    }
    in_maps = []
    for c in range(N_CORES):
        sl = slice(c * BPC, (c + 1) * BPC)
        m = dict(shared)
        m["qT"] = qT[sl]
        m["kT"] = kT[sl]
        m["vT"] = vT[sl]
        m["mb"] = mb[sl]
        in_maps.append(m)
    return in_maps, keff


def kernel(**inputs):
    global _LAST_EXEC_NS, _LAST_RES
    in_maps, keff = _host_prep(inputs)
    nc = _build_program(BPC, keff)
    trace = bool(int(os.environ.get("KERNEL_PROFILE", "0")))
    res = run_bass_kernel_spmd(nc, in_maps, list(range(N_CORES)), trace=trace)
    _LAST_EXEC_NS = res.exec_time_ns
    _LAST_RES = res

    attn = np.empty((B, TQ, TK), dtype=np.float32)
    out = np.empty((B, TQ, CH), dtype=np.float32)
    for c in range(N_CORES):
        r = res.results[c]
        sl = slice(c * BPC, (c + 1) * BPC)
        attn[sl] = r["attnT"].astype(np.float32).transpose(0, 2, 1)
        out[sl] = r["outT"].astype(np.float32).transpose(0, 2, 1)
    return out, attn


# revision 27
# speedup vs baseline: 1.0032x; 1.0032x over previous
"""Trainium2 Bass kernel for the AttentionLayer problem.

Computation (per batch b):
    keys' = keys + sinenc(text_pos, w=1.385);  query' = query + sinenc(frame_pos, w=1.0)
    q = query' @ Wq + bq ; k = keys' @ Wk + bk ; v = values @ Wv + bv
    scores = q @ k^T ; masked softmax over keys -> attn  (output 1)
    out = (attn @ v) * sqrt(1/512) @ Wo + bo             (output 2)

Device strategy: data-parallel over B=64 across 8 cores (8 batches/core).

Weight folding (the big lever): q and k appear ONLY in scores, and v/x
only flow into out, so adjacent projections fold into single GEMMs:
    scores = q'' @ (Wq Wk^T) @ k''^T   with q'' = query + pe_q, k'' = keys + pe_k
      -> device computes  k~ = Wqk @ k''^T  (k-proj shaped),
         then scores = k~^T-contracted with q'' directly: the entire
         q-projection disappears.  Per-query additive terms
         (q_row.bk, bq.bk) are softmax-invariant and dropped; the
         per-key term bq.(k_row+bk) is a host-computed scalar folded
         into the exp bias alongside the mask.
    out = attn @ (values @ (Wv Wo * sqrt(1/TK))) + (bv@Wo*scale + bo)
      -> Wvo := Wv@Wo*scale folds the v-projection and out-projection
         into one; attn rows sum to 1 so bv folds into the constant
         bias (as before).
PE work per batch drops 163->98 streamed 512-col matmul units (-40%).
Numerics (host sim vs f64 reference): out 2.5e-3 / attn 3.6e-3
rel-to-max -- slightly BETTER than the unfolded fp16 scheme (fewer
quantization points), vs the 2e-2 gate.

Transposed layout ([feature, time]) throughout; no on-device transposes:
    k~T[eq,k] = WqkT^T @ keysT + Kpe    (Kpe = ((pe_k@Wk+bk)@Wq^T)^T, host)
    v~T[k,c]  = valuesT^T-slices @ Wvo
    scoresT[k,q] = k~T^T @ q''T          exp via ACT(Exp, bias=mask+ck)
    denom[q] = ones^T @ (tree-summed exp)    attnT = expT * (1/denom)
    outT[c,q] = v~^T @ attnT + bo'       (single GEMM, bias in the drain)

Dtype: all matmuls fp16 (e5m10), f32 PSUM; exp/denominator in f32
(scores reach +62, exp ~1e27).  fp8 was measured (host sim) at 0.4-0.9
rel err on either path -- unusable.  attn and out are DMA'd as fp16 and
widened on host.

Pipelining: per-batch phases issue as
    v~proj(b) | k~proj(b) | sums(b-1) | scores+exp(b) | x+out(b-1)
so the exp->tree-sum chain of b-1 is covered by ~7us of projection
matmuls and the normalized attn tiles are ready before x(b-1) needs
them.  Warmup matmuls run on a memset tile (no DMA dependency) so the
PE p-state ramp is paid during the initial input DMAs.  The final
batch's out DMAs fan out across all four engine queues to shorten the
drain tail.
"""

import math
import os

import numpy as np

import concourse.tile as tile
from concourse import bacc, mybir
from concourse.bass_utils import run_bass_kernel_spmd

dt = mybir.dt
F32 = dt.float32
F32R = dt.float32r
FP16 = dt.float16
AF = mybir.ActivationFunctionType

B, TQ, TK = 64, 1024, 512
CH = 512          # conv_channels == embed_dim == att_hid
N_CORES = 8
BPC = B // N_CORES  # batches per core
KEY_POS_RATE = 1.385
QUERY_POS_RATE = 1.0
OUT_SCALE = math.sqrt(1.0 / TK)
MASK_NEG = -1.0e30
N_WARMUP = 64

_LAST_EXEC_NS = None
_LAST_RES = None


def _sin_pos_enc(pos, w, d):
    """Reference-exact sinusoidal table for one position vector. [T, d] f32."""
    pos = pos.astype(np.float32)
    i = np.arange(d)
    inv_freq = np.power(np.float32(10000.0), -(2.0 * (i // 2)).astype(np.float32) / d)
    ang = (pos * np.float32(w))[:, None] * inv_freq[None, :]
    pe = np.where(i[None, :] % 2 == 0, np.sin(ang), np.cos(ang)).astype(np.float32)
    pe[pos == 0] = 0.0
    return pe


def _build_program(n_batch, keff=TK):
    nc = bacc.Bacc("TRN2", target_bir_lowering=False, debug=False, num_devices=1)

    qT_d = nc.dram_tensor("qT", [n_batch, CH, TQ], FP16, kind="ExternalInput")
    kT_d = nc.dram_tensor("kT", [n_batch, CH, TK], FP16, kind="ExternalInput")
    vT_d = nc.dram_tensor("vT", [n_batch, CH, TK], FP16, kind="ExternalInput")
    kpe_d = nc.dram_tensor("kpe", [CH, TK], FP16, kind="ExternalInput")
    wqk_d = nc.dram_tensor("wqk", [CH, CH], FP16, kind="ExternalInput")
    wvo_d = nc.dram_tensor("wvo", [CH, CH], FP16, kind="ExternalInput")
    bo_d = nc.dram_tensor("bo", [128, CH], F32, kind="ExternalInput")
    mb_d = nc.dram_tensor("mb", [n_batch, TK], F32, kind="ExternalInput")
    ones_d = nc.dram_tensor("ones", [128, 128], F32R, kind="ExternalInput")

    attn_d = nc.dram_tensor("attnT", [n_batch, TK, TQ], FP16, kind="ExternalOutput")
    out_d = nc.dram_tensor("outT", [n_batch, CH, TQ], FP16, kind="ExternalOutput")

    NQ2 = TQ // 512                   # 2 query chunks of 512
    NCT = CH // 128                   # 4 feature tiles
    NKT = TK // 128                   # 4 key tiles
    s512 = lambda c: slice(c * 512, (c + 1) * 512)
    s128 = lambda t: slice(t * 128, (t + 1) * 128)

    with tile.TileContext(nc) as tc:
        with (
            tc.tile_pool(name="wpool", bufs=1) as wpool,
            tc.tile_pool(name="qin", bufs=3) as p_qin,
            tc.tile_pool(name="kin", bufs=3) as p_kin,
            tc.tile_pool(name="vin", bufs=3) as p_vin,
            tc.tile_pool(name="kt", bufs=5) as p_kt,
            tc.tile_pool(name="vt", bufs=8) as p_vt,
            tc.tile_pool(name="vt7", bufs=4) as p_vt7,
            tc.tile_pool(name="exp", bufs=5) as p_exp,
            tc.tile_pool(name="esum", bufs=1) as p_esum,
            tc.tile_pool(name="rec", bufs=2) as p_rec,
            tc.tile_pool(name="attnb", bufs=3) as p_attnb,
            tc.tile_pool(name="outt", bufs=2) as p_out,
            tc.tile_pool(name="ps", bufs=7, space="PSUM") as p_ps,
            tc.tile_pool(name="psw", bufs=1, space="PSUM") as p_ps_warm,
        ):
            ps_one = lambda nm: p_ps.tile([128, 512], F32, name=nm, tag="ps")

            # ---- warmup: lift the PE p-state during initial DMAs.
            # memset (ScalarE) instead of a DMA'd table so the first
            # warmup matmul issues ~us after kernel start, before the
            # DMA rings have even initialized. ----
            warm = wpool.tile([128, 128], FP16, name="warm")
            nc.vector.memset(warm[:], 1.0)
            for i in range(N_WARMUP):
                ps = ps_one(f"warm{i}")
                nc.tensor.matmul(
                    ps[:, :128], warm[:], warm[:],
                    start=True, stop=True,
                )

            # ---- resident weights/constants ----
            # ONE coalesced descriptor per tensor: each dma_start costs
            # ~0.7-1us of sequencer issue time (DIRECT2D), and the serial
            # issue stream -- not DMA bandwidth -- paces the lead-in.
            # sync queue carries the critical path: wvo + batch-0 inputs.
            def load_w(name, dram, dtype, engine=nc.sync):
                t = wpool.tile([128, NCT, CH], dtype, name=name)
                engine.dma_start(
                    t[:], dram.ap().rearrange("(c p) e -> p c e", p=128)
                )
                return [t[:, ct, :] for ct in range(NCT)]

            wqk_w = load_w("wqk", wqk_d, FP16)

            # one-time tables on the gpsimd queue (sync stays free for
            # batch-0 inputs).  bo is pre-broadcast to [128, CH] on host
            # and folded into v~ at the vproj drain (attn rows sum to 1,
            # so adding bo to every v~ row is exact) -- the x drains
            # become plain copies.
            bo_sb = wpool.tile([128, CH], F32, name="bob")
            nc.gpsimd.dma_start(bo_sb[:], bo_d.ap())
            # exp bias (mask + per-key score bias) for all batches in one
            # transfer: col b*NKT+j holds mb[b, j*128:(j+1)*128]
            mb_all = wpool.tile([128, n_batch * NKT], F32, name="mball")
            nc.gpsimd.dma_start(
                mb_all[:], mb_d.ap().rearrange("b (j p) -> p (b j)", p=128)
            )
            ones_sb = wpool.tile([128, 128], F32R, name="ones")
            nc.gpsimd.dma_start(ones_sb[:], ones_d.ap())
            kpe_sb = []  # loaded lazily after batch-0 inputs (first use
            # is the kproj drain, ~7us in; keeps the lead-in DMA window
            # free for the critical first-batch tiles)

            state = {}

            def load_in(pool, dram, b, width, name, engine):
                t = pool.tile([128, NCT, width], FP16, name=name, tag=name[:3])
                engine.dma_start(
                    t[:], dram.ap()[b].rearrange("(c p) x -> p c x", p=128)
                )
                return [t[:, ct, :] for ct in range(NCT)]

            # ALL loads on the sync DGE queue, issued in first-use
            # order: a single queue delivers strictly FIFO, so the
            # first-needed tensor completes after exactly its own bytes.
            # (Measured: splitting across two queues makes the rings
            # interleave both streams and the first tensor arrives at
            # the fair-share point -- 7us later.)
            def load_qin(b):
                return load_in(p_qin, qT_d, b, TQ, f"qin{b}", nc.sync)

            def load_kin(b):
                return load_in(p_kin, kT_d, b, TK, f"kin{b}", nc.sync)

            def load_vin(b):
                return load_in(p_vin, vT_d, b, TK, f"vin{b}", nc.sync)

            def kv_part(b, last=False):
                # ---- inputs, in first-use order (kproj leads: its
                # weights+inputs are the first 1MB on the wire, so the
                # PE gets ~3.5us of kproj cover while vin streams) ----
                kin = load_kin(b)
                if state.get("wvo") is None:
                    state["wvo"] = load_w("wvo", wvo_d, FP16)
                vin = load_vin(b)
                qin = load_qin(b)
                if not kpe_sb:
                    t = wpool.tile([128, NCT, TK], FP16, name="kpe")
                    nc.gpsimd.dma_start(
                        t[:], kpe_d.ap().rearrange("(c p) k -> p c k", p=128)
                    )
                    kpe_sb.extend(t[:, ct, :] for ct in range(NCT))
                wvo = state["wvo"]

                # ---- k~ = Wqk @ keys''T (+ table): [eq, k] tiles ----
                # masked trailing keys (cols keff:TK) never reach the
                # output: their exp is forced to exact 0 by the -1e30
                # bias, so k~ for them may hold stale data -- stream only
                # the first keff columns
                kt = []
                for ht in range(NCT):
                    ps = ps_one(f"psk{b}_{ht}")
                    for ct in range(NCT):
                        nc.tensor.matmul(
                            ps[:, :keff], wqk_w[ct][:, s128(ht)],
                            kin[ct][:, :keff],
                            start=(ct == 0), stop=(ct == NCT - 1),
                        )
                    t = p_kt.tile([128, TK], FP16, name=f"kt{b}_{ht}", tag="kt")
                    if keff < TK and b < 2:
                        # first touch of each of the 5 pool buffers: the
                        # stale-col region must be finite (NaN would
                        # poison exp despite the -1e30 bias)
                        nc.vector.memset(t[:, keff:], 0.0)
                    nc.vector.tensor_add(
                        t[:, :keff], ps[:, :keff], kpe_sb[ht][:, :keff]
                    )
                    kt.append(t)

                if b == 0:
                    # fill the wvo/vin DMA window (~4us) with dummy
                    # matmuls: keeps the p-state ramped so vproj starts
                    # at full clock instead of paying the 3us mid-clock
                    # ramp after an idle gap.  Dedicated psum tag: the
                    # rotating "ps" tag would hand these fills the kproj
                    # banks, whose drains wait on the kpe DMA -- the PE
                    # would stall inside the fills.
                    for i in range(12):
                        ps = p_ps_warm.tile(
                            [128, 128], F32, name=f"fill{i}", tag="warmps"
                        )
                        nc.tensor.matmul(
                            ps[:], warm[:], warm[:],
                            start=True, stop=True,
                        )

                # ---- v~ = values @ (Wv Wo * scale) + bo: [k, c] tiles.
                # The last batch's v~ is kept f32r so its x-phase can
                # consume the unnormalized exp tiles directly. ----
                vt = []
                for ktile in range(NKT):
                    ps = ps_one(f"psv{b}_{ktile}")
                    for ct in range(NCT):
                        nc.tensor.matmul(
                            ps[:], vin[ct][:, s128(ktile)], wvo[ct][:],
                            start=(ct == 0), stop=(ct == NCT - 1),
                        )
                    if last:
                        t = p_vt7.tile([128, CH], F32R, name=f"vt7_{ktile}",
                                       tag="vt7")
                    else:
                        t = p_vt.tile([128, CH], FP16, name=f"vt{b}_{ktile}",
                                      tag="vt")
                    nc.vector.tensor_add(t[:], ps[:], bo_sb[:])
                    vt.append(t)
                return qin, kt, vt

            def scores_exp(b, qt, kt):
                # ---- scores + exp (mask & key-bias folded into bias) ----
                expt = []
                for ktile in range(NKT):
                    ps = [ps_one(f"pss{b}_{ktile}_{c}") for c in range(NQ2)]
                    for ht in range(NCT):
                        for c in range(NQ2):
                            nc.tensor.matmul(
                                ps[c][:], kt[ht][:, s128(ktile)],
                                qt[ht][:, s512(c)],
                                start=(ht == 0), stop=(ht == NCT - 1),
                            )
                    t = p_exp.tile([128, TQ], F32R, name=f"exp{b}_{ktile}", tag="exp")
                    for c in range(NQ2):
                        nc.scalar.activation(
                            t[:, s512(c)], ps[c][:], AF.Exp,
                            bias=mb_all[:, b * NKT + ktile:b * NKT + ktile + 1],
                        )
                    expt.append(t)
                return expt

            def sums_recip(b, expt):
                # tree-sum the 4 exp tiles on GpSimd, then 2 ones-matmuls
                s01 = p_esum.tile([128, TQ], F32R, name=f"s01_{b}", tag="esA")
                s23 = p_esum.tile([128, TQ], F32R, name=f"s23_{b}", tag="esB")
                s = p_esum.tile([128, TQ], F32R, name=f"s_{b}", tag="esC")
                nc.gpsimd.tensor_add(s01[:], expt[0][:], expt[1][:])
                nc.gpsimd.tensor_add(s23[:], expt[2][:], expt[3][:])
                nc.gpsimd.tensor_add(s[:], s01[:], s23[:])
                rec = p_rec.tile([128, TQ], F32, name=f"rec{b}", tag="rec")
                for c in range(NQ2):
                    ps = ps_one(f"pssum{b}_{c}")
                    nc.tensor.matmul(
                        ps[:], ones_sb[:], s[:, s512(c)],
                        start=True, stop=True,
                    )
                    nc.vector.reciprocal_approx_fast(rec[:, s512(c)], ps[:])
                return rec

            def attn_norm(b, expt, rec):
                # one big [128, NKT*TQ] tile, one output descriptor
                t = p_attnb.tile(
                    [128, NKT, TQ], FP16, name=f"ab{b}", tag="attnb"
                )
                attn = [t[:, ktile, :] for ktile in range(NKT)]
                for ktile in range(NKT):
                    for c in range(NQ2):
                        nc.vector.tensor_mul(
                            attn[ktile][:, s512(c)], expt[ktile][:, s512(c)],
                            rec[:, s512(c)],
                        )
                nc.gpsimd.dma_start(
                    attn_d.ap()[b].rearrange("(j p) q -> p j q", p=128), t[:]
                )
                return attn

            def x_out(b, vt, attn):
                # out = attn @ (v~ + bo) in one GEMM; drain is a plain
                # copy into one big tile, one output descriptor
                t = p_out.tile([128, NCT, TQ], FP16, name=f"ot{b}", tag="outt")
                for ct in range(NCT):
                    ps = [ps_one(f"psx{b}_{ct}_{c}") for c in range(NQ2)]
                    for ktile in range(NKT):
                        for c in range(NQ2):
                            nc.tensor.matmul(
                                ps[c][:], vt[ktile][:, s128(ct)],
                                attn[ktile][:, s512(c)],
                                start=(ktile == 0), stop=(ktile == NKT - 1),
                            )
                    for c in range(NQ2):
                        # alternate psum-drain engines so the 8 drains don't
                        # serialize on one engine
                        dst = t[:, ct, s512(c)]
                        if ct % 2 == 0:
                            nc.scalar.copy(dst, ps[c][:])
                        else:
                            nc.vector.tensor_copy(dst, ps[c][:])
                nc.gpsimd.dma_start(
                    out_d.ap()[b].rearrange("(c p) q -> p c q", p=128), t[:]
                )

            def x_out_last(b, vt, expt, rec):
                # Final batch: consume UNNORMALIZED exp tiles (f32r) and
                # fold 1/denom into the drain multiply, so the x matmuls
                # start right after the sums matmuls with no wait on
                # attn-norm.  rec columns are partition-replicated, so the
                # drain is a single tensor-tensor multiply per chunk.
                # attn-norm muls interleave here, split Vector/GpSimd, so
                # no single engine serializes the endgame; attn DMAs flush
                # per-tile during the x matmuls.
                at = p_attnb.tile(
                    [128, NKT, TQ], FP16, name=f"ab{b}", tag="attnb"
                )
                dma_engines = [nc.gpsimd, nc.scalar, nc.sync]
                for ct in range(NCT):
                    ps = [ps_one(f"psx{b}_{ct}_{c}") for c in range(NQ2)]
                    for ktile in range(NKT):
                        for c in range(NQ2):
                            nc.tensor.matmul(
                                ps[c][:], vt[ktile][:, s128(ct)],
                                expt[ktile][:, s512(c)],
                                start=(ktile == 0), stop=(ktile == NKT - 1),
                            )
                    # attn tile `ct`: vector does chunk 0, gpsimd chunk 1
                    asl = at[:, ct, :]
                    nc.vector.tensor_mul(
                        asl[:, s512(0)], expt[ct][:, s512(0)], rec[:, s512(0)]
                    )
                    nc.gpsimd.tensor_mul(
                        asl[:, s512(1)], expt[ct][:, s512(1)], rec[:, s512(1)]
                    )
                    nc.gpsimd.dma_start(attn_d.ap()[b, s128(ct), :], asl)
                    t = p_out.tile([128, TQ], FP16, name=f"ot7_{ct}", tag="out7")
                    for c in range(NQ2):
                        nc.vector.tensor_mul(
                            t[:, s512(c)], ps[c][:], rec[:, s512(c)]
                        )
                        # chunked final store across three engine queues:
                        # start draining sooner, flush faster
                        eng = dma_engines[(ct * NQ2 + c) % 3]
                        eng.dma_start(
                            out_d.ap()[b, s128(ct), s512(c)], t[:, s512(c)]
                        )

            # 2-deep pipeline: per iteration
            #   v~proj(b) k~proj(b) | sums(b-1) | scores+exp(b) | x+out(b-1)
            # exp(b-1)'s tree-sum is covered by both projections (~7us);
            # recip+attn-norm(b-1) by scores(b).
            prev = None  # (expt, vt) of previous batch
            for b in range(n_batch):
                qin, kt, vt = kv_part(b, last=(b == n_batch - 1))
                if prev is not None:
                    rec = sums_recip(b - 1, prev[0])
                    attn = attn_norm(b - 1, prev[0], rec)
                expt = scores_exp(b, qin, kt)
                if prev is not None:
                    x_out(b - 1, prev[1], attn)
                prev = (expt, vt)
            b = n_batch - 1
            rec = sums_recip(b, prev[0])
            x_out_last(b, prev[1], prev[0], rec)
    nc.compile()
    return nc


def _host_prep(inputs):
    query = np.asarray(inputs["query"], dtype=np.float32)
    keys = np.asarray(inputs["keys"], dtype=np.float32)
    values = np.asarray(inputs["values"], dtype=np.float32)
    tpos = np.asarray(inputs["text_positions"])
    fpos = np.asarray(inputs["frame_positions"])
    mask = np.asarray(inputs["mask"])
    Wq = np.asarray(inputs["Wq"], dtype=np.float32)
    Wk = np.asarray(inputs["Wk"], dtype=np.float32)
    Wv = np.asarray(inputs["Wv"], dtype=np.float32)
    Wo = np.asarray(inputs["Wo"], dtype=np.float32)
    bq = np.asarray(inputs["bq"], dtype=np.float32)
    bk = np.asarray(inputs["bk"], dtype=np.float32)
    bv = np.asarray(inputs["bv"], dtype=np.float32)
    bo = np.asarray(inputs["bo"], dtype=np.float32)

    fshared = bool(np.all(fpos == fpos[0:1]))
    tshared = bool(np.all(tpos == tpos[0:1]))

    # q'' = query + pe_q, quantized once (no q-projection on device)
    if fshared:
        peq = _sin_pos_enc(fpos[0], QUERY_POS_RATE, CH)       # [TQ, CH]
        qT = (query + peq[None]).transpose(0, 2, 1)
    else:
        peq = np.stack([_sin_pos_enc(p, QUERY_POS_RATE, CH) for p in fpos])
        qT = (query + peq).transpose(0, 2, 1)

    # k'' = keys + pe_k; pe routed through the f32 table when shared
    wkq = Wk @ Wq.T                                            # [ek, eq]
    if tshared:
        pek = _sin_pos_enc(tpos[0], KEY_POS_RATE, CH)          # [TK, CH]
        kT = keys.transpose(0, 2, 1)
        kpe = np.ascontiguousarray(((pek @ Wk + bk) @ Wq.T).T)  # [eq, TK]
        keys_full = keys + pek[None]
    else:
        pek = np.stack([_sin_pos_enc(p, KEY_POS_RATE, CH) for p in tpos])
        kT = (keys + pek).transpose(0, 2, 1)
        kpe = np.ascontiguousarray(
            np.broadcast_to((bk @ Wq.T)[:, None], (CH, TK))
        ).astype(np.float32)
        keys_full = keys + pek

    qT = np.ascontiguousarray(qT).astype(np.float16)
    kT = np.ascontiguousarray(kT).astype(np.float16)
    vT = np.ascontiguousarray(values.transpose(0, 2, 1)).astype(np.float16)

    # exp bias: mask plus the per-key bq.(k_row+bk) term the q-side
    # folding moved out of the scores GEMM (zero when bq == 0)
    ck = keys_full @ (Wk @ bq) + np.float32(bk @ bq)           # [B, TK]
    mb = np.where(mask, np.float32(MASK_NEG), np.float32(0.0)) + ck
    mb = mb.astype(np.float32)
    ones = np.ones((128, 128), dtype=np.float32)

    # masked-tail fast path: if every batch masks exactly the last 64
    # keys, the k-projection can skip those columns (their exp is forced
    # to exact zero by the bias regardless of the k~ values there)
    keff = TK
    if bool(np.array_equal(
        mask, np.broadcast_to(np.arange(TK) >= TK - 64, mask.shape)
    )):
        keff = TK - 64

    bo2 = (np.float32(OUT_SCALE) * (bv @ Wo) + bo).astype(np.float32)
    shared = {
        "wqk": wkq.astype(np.float16),
        "wvo": (Wv @ Wo * np.float32(OUT_SCALE)).astype(np.float16),
        "kpe": kpe.astype(np.float16),
        "bo": np.ascontiguousarray(np.broadcast_to(bo2[None, :], (128, CH))),
        "ones": ones,
    }
    in_maps = []
    for c in range(N_CORES):
        sl = slice(c * BPC, (c + 1) * BPC)
        m = dict(shared)
        m["qT"] = qT[sl]
        m["kT"] = kT[sl]
        m["vT"] = vT[sl]
        m["mb"] = mb[sl]
        in_maps.append(m)
    return in_maps, keff


def kernel(**inputs):
    global _LAST_EXEC_NS, _LAST_RES
    in_maps, keff = _host_prep(inputs)
    nc = _build_program(BPC, keff)
    trace = bool(int(os.environ.get("KERNEL_PROFILE", "0")))
    res = run_bass_kernel_spmd(nc, in_maps, list(range(N_CORES)), trace=trace)
    _LAST_EXEC_NS = res.exec_time_ns
    _LAST_RES = res

    attn = np.empty((B, TQ, TK), dtype=np.float32)
    out = np.empty((B, TQ, CH), dtype=np.float32)
    for c in range(N_CORES):
        r = res.results[c]
        sl = slice(c * BPC, (c + 1) * BPC)
        attn[sl] = r["attnT"].astype(np.float32).transpose(0, 2, 1)
        out[sl] = r["outT"].astype(np.float32).transpose(0, 2, 1)
    return out, attn


# revision 28
# speedup vs baseline: 1.0182x; 1.0150x over previous
"""Trainium2 Bass kernel for the AttentionLayer problem.

Computation (per batch b):
    keys' = keys + sinenc(text_pos, w=1.385);  query' = query + sinenc(frame_pos, w=1.0)
    q = query' @ Wq + bq ; k = keys' @ Wk + bk ; v = values @ Wv + bv
    scores = q @ k^T ; masked softmax over keys -> attn  (output 1)
    out = (attn @ v) * sqrt(1/512) @ Wo + bo             (output 2)

Device strategy: data-parallel over B=64 across 8 cores (8 batches/core).

Weight folding (the big lever): q and k appear ONLY in scores, and v/x
only flow into out, so adjacent projections fold into single GEMMs:
    scores = q'' @ (Wq Wk^T) @ k''^T   with q'' = query + pe_q, k'' = keys + pe_k
      -> device computes  k~ = Wqk @ k''^T  (k-proj shaped),
         then scores = k~^T-contracted with q'' directly: the entire
         q-projection disappears.  Per-query additive terms
         (q_row.bk, bq.bk) are softmax-invariant and dropped; the
         per-key term bq.(k_row+bk) is a host-computed scalar folded
         into the exp bias alongside the mask.
    out = attn @ (values @ (Wv Wo * sqrt(1/TK))) + (bv@Wo*scale + bo)
      -> Wvo := Wv@Wo*scale folds the v-projection and out-projection
         into one; attn rows sum to 1 so bv folds into the constant
         bias (as before).
PE work per batch drops 163->98 streamed 512-col matmul units (-40%).
Numerics (host sim vs f64 reference): out 2.5e-3 / attn 3.6e-3
rel-to-max -- slightly BETTER than the unfolded fp16 scheme (fewer
quantization points), vs the 2e-2 gate.

Transposed layout ([feature, time]) throughout; no on-device transposes:
    k~T[eq,k] = WqkT^T @ keysT + Kpe    (Kpe = ((pe_k@Wk+bk)@Wq^T)^T, host)
    v~T[k,c]  = valuesT^T-slices @ Wvo
    scoresT[k,q] = k~T^T @ q''T          exp via ACT(Exp, bias=mask+ck)
    denom[q] = ones^T @ (tree-summed exp)    attnT = expT * (1/denom)
    outT[c,q] = v~^T @ attnT + bo'       (single GEMM, bias in the drain)

Dtype: all matmuls fp16 (e5m10), f32 PSUM; exp/denominator in f32
(scores reach +62, exp ~1e27).  fp8 was measured (host sim) at 0.4-0.9
rel err on either path -- unusable.  attn and out are DMA'd as fp16 and
widened on host.

Pipelining: per-batch phases issue as
    v~proj(b) | k~proj(b) | sums(b-1) | scores+exp(b) | x+out(b-1)
so the exp->tree-sum chain of b-1 is covered by ~7us of projection
matmuls and the normalized attn tiles are ready before x(b-1) needs
them.  Warmup matmuls run on a memset tile (no DMA dependency) so the
PE p-state ramp is paid during the initial input DMAs.  The final
batch's out DMAs fan out across all four engine queues to shorten the
drain tail.
"""

import math
import os

import numpy as np

import concourse.tile as tile
from concourse import bacc, mybir
from concourse.bass_utils import run_bass_kernel_spmd

dt = mybir.dt
F32 = dt.float32
F32R = dt.float32r
FP16 = dt.float16
AF = mybir.ActivationFunctionType

B, TQ, TK = 64, 1024, 512
CH = 512          # conv_channels == embed_dim == att_hid
N_CORES = 8
BPC = B // N_CORES  # batches per core
KEY_POS_RATE = 1.385
QUERY_POS_RATE = 1.0
OUT_SCALE = math.sqrt(1.0 / TK)
MASK_NEG = -1.0e30
N_WARMUP = 64

_LAST_EXEC_NS = None
_LAST_RES = None


def _sin_pos_enc(pos, w, d):
    """Reference-exact sinusoidal table for one position vector. [T, d] f32."""
    pos = pos.astype(np.float32)
    i = np.arange(d)
    inv_freq = np.power(np.float32(10000.0), -(2.0 * (i // 2)).astype(np.float32) / d)
    ang = (pos * np.float32(w))[:, None] * inv_freq[None, :]
    pe = np.where(i[None, :] % 2 == 0, np.sin(ang), np.cos(ang)).astype(np.float32)
    pe[pos == 0] = 0.0
    return pe


def _build_program(n_batch, keff=TK):
    nc = bacc.Bacc("TRN2", target_bir_lowering=False, debug=False, num_devices=1)

    qT_d = nc.dram_tensor("qT", [n_batch, CH, TQ], FP16, kind="ExternalInput")
    kT_d = nc.dram_tensor("kT", [n_batch, CH, TK], FP16, kind="ExternalInput")
    vT_d = nc.dram_tensor("vT", [n_batch, CH, TK], FP16, kind="ExternalInput")
    kpe_d = nc.dram_tensor("kpe", [CH, TK], FP16, kind="ExternalInput")
    wqk_d = nc.dram_tensor("wqk", [CH, CH], FP16, kind="ExternalInput")
    wvo_d = nc.dram_tensor("wvo", [CH, CH], FP16, kind="ExternalInput")
    bo_d = nc.dram_tensor("bo", [128, CH], F32, kind="ExternalInput")
    mb_d = nc.dram_tensor("mb", [n_batch, TK], F32, kind="ExternalInput")
    ones_d = nc.dram_tensor("ones", [128, 128], F32R, kind="ExternalInput")

    attn_d = nc.dram_tensor("attnT", [n_batch, TK, TQ], FP16, kind="ExternalOutput")
    out_d = nc.dram_tensor("outT", [n_batch, CH, TQ], FP16, kind="ExternalOutput")

    NQ2 = TQ // 512                   # 2 query chunks of 512
    NCT = CH // 128                   # 4 feature tiles
    NKT = TK // 128                   # 4 key tiles
    s512 = lambda c: slice(c * 512, (c + 1) * 512)
    s128 = lambda t: slice(t * 128, (t + 1) * 128)

    with tile.TileContext(nc) as tc:
        with (
            tc.tile_pool(name="wpool", bufs=1) as wpool,
            tc.tile_pool(name="qin", bufs=3) as p_qin,
            tc.tile_pool(name="kin", bufs=3) as p_kin,
            tc.tile_pool(name="vin", bufs=3) as p_vin,
            tc.tile_pool(name="kt", bufs=5) as p_kt,
            tc.tile_pool(name="vt", bufs=8) as p_vt,
            tc.tile_pool(name="vt7", bufs=4) as p_vt7,
            tc.tile_pool(name="exp", bufs=5) as p_exp,
            tc.tile_pool(name="esum", bufs=1) as p_esum,
            tc.tile_pool(name="rec", bufs=2) as p_rec,
            tc.tile_pool(name="attnb", bufs=3) as p_attnb,
            tc.tile_pool(name="outt", bufs=2) as p_out,
            tc.tile_pool(name="ps", bufs=8, space="PSUM") as p_ps,
        ):
            ps_one = lambda nm: p_ps.tile([128, 512], F32, name=nm, tag="ps")

            # ---- warmup: lift the PE p-state during initial DMAs.
            # memset (ScalarE) instead of a DMA'd table so the first
            # warmup matmul issues ~us after kernel start, before the
            # DMA rings have even initialized. ----
            warm = wpool.tile([128, 128], FP16, name="warm")
            nc.vector.memset(warm[:], 1.0)
            for i in range(N_WARMUP):
                ps = ps_one(f"warm{i}")
                nc.tensor.matmul(
                    ps[:, :128], warm[:], warm[:],
                    start=True, stop=True,
                )

            # ---- resident weights/constants ----
            # ONE coalesced descriptor per tensor: each dma_start costs
            # ~0.7-1us of sequencer issue time (DIRECT2D), and the serial
            # issue stream -- not DMA bandwidth -- paces the lead-in.
            # sync queue carries the critical path: wvo + batch-0 inputs.
            def load_w(name, dram, dtype, engine=nc.sync):
                t = wpool.tile([128, NCT, CH], dtype, name=name)
                engine.dma_start(
                    t[:], dram.ap().rearrange("(c p) e -> p c e", p=128)
                )
                return [t[:, ct, :] for ct in range(NCT)]

            wqk_w = load_w("wqk", wqk_d, FP16)

            # one-time tables on the gpsimd queue (sync stays free for
            # batch-0 inputs).  bo is pre-broadcast to [128, CH] on host
            # and folded into v~ at the vproj drain (attn rows sum to 1,
            # so adding bo to every v~ row is exact) -- the x drains
            # become plain copies.
            bo_sb = wpool.tile([128, CH], F32, name="bob")
            nc.gpsimd.dma_start(bo_sb[:], bo_d.ap())
            # exp bias (mask + per-key score bias) for all batches in one
            # transfer: col b*NKT+j holds mb[b, j*128:(j+1)*128]
            mb_all = wpool.tile([128, n_batch * NKT], F32, name="mball")
            nc.gpsimd.dma_start(
                mb_all[:], mb_d.ap().rearrange("b (j p) -> p (b j)", p=128)
            )
            ones_sb = wpool.tile([128, 128], F32R, name="ones")
            nc.gpsimd.dma_start(ones_sb[:], ones_d.ap())
            kpe_sb = []  # loaded lazily after batch-0 inputs (first use
            # is the kproj drain, ~7us in; keeps the lead-in DMA window
            # free for the critical first-batch tiles)

            state = {}

            def load_in(pool, dram, b, width, name, engine):
                t = pool.tile([128, NCT, width], FP16, name=name, tag=name[:3])
                engine.dma_start(
                    t[:], dram.ap()[b].rearrange("(c p) x -> p c x", p=128)
                )
                return [t[:, ct, :] for ct in range(NCT)]

            # ALL loads on the sync DGE queue, issued in first-use
            # order: a single queue delivers strictly FIFO, so the
            # first-needed tensor completes after exactly its own bytes.
            # (Measured: splitting across two queues makes the rings
            # interleave both streams and the first tensor arrives at
            # the fair-share point -- 7us later.)
            def load_qin(b):
                return load_in(p_qin, qT_d, b, TQ, f"qin{b}", nc.sync)

            def load_kin(b):
                return load_in(p_kin, kT_d, b, TK, f"kin{b}", nc.sync)

            def load_vin(b):
                return load_in(p_vin, vT_d, b, TK, f"vin{b}", nc.sync)

            def kv_part(b, last=False):
                # ---- inputs, in first-use order (kproj leads: its
                # weights+inputs are the first 1MB on the wire, so the
                # PE gets ~3.5us of kproj cover while vin streams) ----
                kin = load_kin(b)
                if state.get("wvo") is None:
                    state["wvo"] = load_w("wvo", wvo_d, FP16)
                vin = load_vin(b)
                qin = load_qin(b)
                if not kpe_sb:
                    t = wpool.tile([128, NCT, TK], FP16, name="kpe")
                    nc.gpsimd.dma_start(
                        t[:], kpe_d.ap().rearrange("(c p) k -> p c k", p=128)
                    )
                    kpe_sb.extend(t[:, ct, :] for ct in range(NCT))
                wvo = state["wvo"]

                # ---- k~ = Wqk @ keys''T (+ table): [eq, k] tiles ----
                # masked trailing keys (cols keff:TK) never reach the
                # output: their exp is forced to exact 0 by the -1e30
                # bias, so k~ for them may hold stale data -- stream only
                # the first keff columns
                kt = []
                for ht in range(NCT):
                    ps = ps_one(f"psk{b}_{ht}")
                    for ct in range(NCT):
                        nc.tensor.matmul(
                            ps[:, :keff], wqk_w[ct][:, s128(ht)],
                            kin[ct][:, :keff],
                            start=(ct == 0), stop=(ct == NCT - 1),
                        )
                    t = p_kt.tile([128, TK], FP16, name=f"kt{b}_{ht}", tag="kt")
                    if keff < TK and b < 2:
                        # first touch of each of the 5 pool buffers: the
                        # stale-col region must be finite (NaN would
                        # poison exp despite the -1e30 bias)
                        nc.vector.memset(t[:, keff:], 0.0)
                    nc.vector.tensor_add(
                        t[:, :keff], ps[:, :keff], kpe_sb[ht][:, :keff]
                    )
                    kt.append(t)


                # ---- v~ = values @ (Wv Wo * scale) + bo: [k, c] tiles.
                # The last batch's v~ is kept f32r so its x-phase can
                # consume the unnormalized exp tiles directly. ----
                vt = []
                for ktile in range(NKT):
                    ps = ps_one(f"psv{b}_{ktile}")
                    for ct in range(NCT):
                        nc.tensor.matmul(
                            ps[:], vin[ct][:, s128(ktile)], wvo[ct][:],
                            start=(ct == 0), stop=(ct == NCT - 1),
                        )
                    if last:
                        t = p_vt7.tile([128, CH], F32R, name=f"vt7_{ktile}",
                                       tag="vt7")
                    else:
                        t = p_vt.tile([128, CH], FP16, name=f"vt{b}_{ktile}",
                                      tag="vt")
                    nc.vector.tensor_add(t[:], ps[:], bo_sb[:])
                    vt.append(t)
                return qin, kt, vt

            def scores_exp(b, qt, kt):
                # ---- scores + exp (mask & key-bias folded into bias) ----
                expt = []
                for ktile in range(NKT):
                    ps = [ps_one(f"pss{b}_{ktile}_{c}") for c in range(NQ2)]
                    for ht in range(NCT):
                        for c in range(NQ2):
                            nc.tensor.matmul(
                                ps[c][:], kt[ht][:, s128(ktile)],
                                qt[ht][:, s512(c)],
                                start=(ht == 0), stop=(ht == NCT - 1),
                            )
                    t = p_exp.tile([128, TQ], F32R, name=f"exp{b}_{ktile}", tag="exp")
                    for c in range(NQ2):
                        nc.scalar.activation(
                            t[:, s512(c)], ps[c][:], AF.Exp,
                            bias=mb_all[:, b * NKT + ktile:b * NKT + ktile + 1],
                        )
                    expt.append(t)
                return expt

            def sums_recip(b, expt):
                # tree-sum the 4 exp tiles on GpSimd, then 2 ones-matmuls
                s01 = p_esum.tile([128, TQ], F32R, name=f"s01_{b}", tag="esA")
                s23 = p_esum.tile([128, TQ], F32R, name=f"s23_{b}", tag="esB")
                s = p_esum.tile([128, TQ], F32R, name=f"s_{b}", tag="esC")
                nc.gpsimd.tensor_add(s01[:], expt[0][:], expt[1][:])
                nc.gpsimd.tensor_add(s23[:], expt[2][:], expt[3][:])
                nc.gpsimd.tensor_add(s[:], s01[:], s23[:])
                rec = p_rec.tile([128, TQ], F32, name=f"rec{b}", tag="rec")
                for c in range(NQ2):
                    ps = ps_one(f"pssum{b}_{c}")
                    nc.tensor.matmul(
                        ps[:], ones_sb[:], s[:, s512(c)],
                        start=True, stop=True,
                    )
                    nc.vector.reciprocal_approx_fast(rec[:, s512(c)], ps[:])
                return rec

            def attn_norm(b, expt, rec):
                # one big [128, NKT*TQ] tile, one output descriptor
                t = p_attnb.tile(
                    [128, NKT, TQ], FP16, name=f"ab{b}", tag="attnb"
                )
                attn = [t[:, ktile, :] for ktile in range(NKT)]
                for ktile in range(NKT):
                    for c in range(NQ2):
                        nc.vector.tensor_mul(
                            attn[ktile][:, s512(c)], expt[ktile][:, s512(c)],
                            rec[:, s512(c)],
                        )
                nc.gpsimd.dma_start(
                    attn_d.ap()[b].rearrange("(j p) q -> p j q", p=128), t[:]
                )
                return attn

            def x_out(b, vt, attn):
                # out = attn @ (v~ + bo) in one GEMM; drain is a plain
                # copy into one big tile, one output descriptor
                t = p_out.tile([128, NCT, TQ], FP16, name=f"ot{b}", tag="outt")
                for ct in range(NCT):
                    ps = [ps_one(f"psx{b}_{ct}_{c}") for c in range(NQ2)]
                    for ktile in range(NKT):
                        for c in range(NQ2):
                            nc.tensor.matmul(
                                ps[c][:], vt[ktile][:, s128(ct)],
                                attn[ktile][:, s512(c)],
                                start=(ktile == 0), stop=(ktile == NKT - 1),
                            )
                    for c in range(NQ2):
                        # alternate psum-drain engines so the 8 drains don't
                        # serialize on one engine
                        dst = t[:, ct, s512(c)]
                        if ct % 2 == 0:
                            nc.scalar.copy(dst, ps[c][:])
                        else:
                            nc.vector.tensor_copy(dst, ps[c][:])
                nc.gpsimd.dma_start(
                    out_d.ap()[b].rearrange("(c p) q -> p c q", p=128), t[:]
                )

            def x_out_last(b, vt, expt, rec):
                # Final batch: consume UNNORMALIZED exp tiles (f32r) and
                # fold 1/denom into the drain multiply, so the x matmuls
                # start right after the sums matmuls with no wait on
                # attn-norm.  rec columns are partition-replicated, so the
                # drain is a single tensor-tensor multiply per chunk.
                # attn-norm muls interleave here, split Vector/GpSimd, so
                # no single engine serializes the endgame; attn DMAs flush
                # per-tile during the x matmuls.
                at = p_attnb.tile(
                    [128, NKT, TQ], FP16, name=f"ab{b}", tag="attnb"
                )
                dma_engines = [nc.gpsimd, nc.scalar, nc.sync]
                for ct in range(NCT):
                    ps = [ps_one(f"psx{b}_{ct}_{c}") for c in range(NQ2)]
                    for ktile in range(NKT):
                        for c in range(NQ2):
                            nc.tensor.matmul(
                                ps[c][:], vt[ktile][:, s128(ct)],
                                expt[ktile][:, s512(c)],
                                start=(ktile == 0), stop=(ktile == NKT - 1),
                            )
                    # attn tile `ct`: vector does chunk 0, gpsimd chunk 1
                    asl = at[:, ct, :]
                    nc.vector.tensor_mul(
                        asl[:, s512(0)], expt[ct][:, s512(0)], rec[:, s512(0)]
                    )
                    nc.gpsimd.tensor_mul(
                        asl[:, s512(1)], expt[ct][:, s512(1)], rec[:, s512(1)]
                    )
                    nc.gpsimd.dma_start(attn_d.ap()[b, s128(ct), :], asl)
                    t = p_out.tile([128, TQ], FP16, name=f"ot7_{ct}", tag="out7")
                    for c in range(NQ2):
                        nc.vector.tensor_mul(
                            t[:, s512(c)], ps[c][:], rec[:, s512(c)]
                        )
                        # chunked final store across three engine queues:
                        # start draining sooner, flush faster
                        eng = dma_engines[(ct * NQ2 + c) % 3]
                        eng.dma_start(
                            out_d.ap()[b, s128(ct), s512(c)], t[:, s512(c)]
                        )

            # 2-deep pipeline: per iteration
            #   v~proj(b) k~proj(b) | sums(b-1) | scores+exp(b) | x+out(b-1)
            # exp(b-1)'s tree-sum is covered by both projections (~7us);
            # recip+attn-norm(b-1) by scores(b).
            prev = None  # (expt, vt) of previous batch
            for b in range(n_batch):
                qin, kt, vt = kv_part(b, last=(b == n_batch - 1))
                if prev is not None:
                    rec = sums_recip(b - 1, prev[0])
                    attn = attn_norm(b - 1, prev[0], rec)
                expt = scores_exp(b, qin, kt)
                if prev is not None:
                    x_out(b - 1, prev[1], attn)
                prev = (expt, vt)
            b = n_batch - 1
            rec = sums_recip(b, prev[0])
            x_out_last(b, prev[1], prev[0], rec)
    nc.compile()
    return nc


def _host_prep(inputs):
    query = np.asarray(inputs["query"], dtype=np.float32)
    keys = np.asarray(inputs["keys"], dtype=np.float32)
    values = np.asarray(inputs["values"], dtype=np.float32)
    tpos = np.asarray(inputs["text_positions"])
    fpos = np.asarray(inputs["frame_positions"])
    mask = np.asarray(inputs["mask"])
    Wq = np.asarray(inputs["Wq"], dtype=np.float32)
    Wk = np.asarray(inputs["Wk"], dtype=np.float32)
    Wv = np.asarray(inputs["Wv"], dtype=np.float32)
    Wo = np.asarray(inputs["Wo"], dtype=np.float32)
    bq = np.asarray(inputs["bq"], dtype=np.float32)
    bk = np.asarray(inputs["bk"], dtype=np.float32)
    bv = np.asarray(inputs["bv"], dtype=np.float32)
    bo = np.asarray(inputs["bo"], dtype=np.float32)

    fshared = bool(np.all(fpos == fpos[0:1]))
    tshared = bool(np.all(tpos == tpos[0:1]))

    # q'' = query + pe_q, quantized once (no q-projection on device)
    if fshared:
        peq = _sin_pos_enc(fpos[0], QUERY_POS_RATE, CH)       # [TQ, CH]
        qT = (query + peq[None]).transpose(0, 2, 1)
    else:
        peq = np.stack([_sin_pos_enc(p, QUERY_POS_RATE, CH) for p in fpos])
        qT = (query + peq).transpose(0, 2, 1)

    # k'' = keys + pe_k; pe routed through the f32 table when shared
    wkq = Wk @ Wq.T                                            # [ek, eq]
    if tshared:
        pek = _sin_pos_enc(tpos[0], KEY_POS_RATE, CH)          # [TK, CH]
        kT = keys.transpose(0, 2, 1)
        kpe = np.ascontiguousarray(((pek @ Wk + bk) @ Wq.T).T)  # [eq, TK]
        keys_full = keys + pek[None]
    else:
        pek = np.stack([_sin_pos_enc(p, KEY_POS_RATE, CH) for p in tpos])
        kT = (keys + pek).transpose(0, 2, 1)
        kpe = np.ascontiguousarray(
            np.broadcast_to((bk @ Wq.T)[:, None], (CH, TK))
        ).astype(np.float32)
        keys_full = keys + pek

    qT = np.ascontiguousarray(qT).astype(np.float16)
    kT = np.ascontiguousarray(kT).astype(np.float16)
    vT = np.ascontiguousarray(values.transpose(0, 2, 1)).astype(np.float16)

    # exp bias: mask plus the per-key bq.(k_row+bk) term the q-side
    # folding moved out of the scores GEMM (zero when bq == 0)
    ck = keys_full @ (Wk @ bq) + np.float32(bk @ bq)           # [B, TK]
    mb = np.where(mask, np.float32(MASK_NEG), np.float32(0.0)) + ck
    mb = mb.astype(np.float32)
    ones = np.ones((128, 128), dtype=np.float32)

    # masked-tail fast path: if every batch masks exactly the last 64
    # keys, the k-projection can skip those columns (their exp is forced
    # to exact zero by the bias regardless of the k~ values there)
    keff = TK
    if bool(np.array_equal(
        mask, np.broadcast_to(np.arange(TK) >= TK - 64, mask.shape)
    )):
        keff = TK - 64

    bo2 = (np.float32(OUT_SCALE) * (bv @ Wo) + bo).astype(np.float32)
    shared = {
        "wqk": wkq.astype(np.float16),
        "wvo": (Wv @ Wo * np.float32(OUT_SCALE)).astype(np.float16),
        "kpe": kpe.astype(np.float16),
        "bo": np.ascontiguousarray(np.broadcast_to(bo2[None, :], (128, CH))),
        "ones": ones,
    }
    in_maps = []
    for c in range(N_CORES):
        sl = slice(c * BPC, (c + 1) * BPC)
        m = dict(shared)
        m["qT"] = qT[sl]
        m["kT"] = kT[sl]
        m["vT"] = vT[sl]
        m["mb"] = mb[sl]
        in_maps.append(m)
    return in_maps, keff


def kernel(**inputs):
    global _LAST_EXEC_NS, _LAST_RES
    in_maps, keff = _host_prep(inputs)
    nc = _build_program(BPC, keff)
    trace = bool(int(os.environ.get("KERNEL_PROFILE", "0")))
    res = run_bass_kernel_spmd(nc, in_maps, list(range(N_CORES)), trace=trace)
    _LAST_EXEC_NS = res.exec_time_ns
    _LAST_RES = res

    attn = np.empty((B, TQ, TK), dtype=np.float32)
    out = np.empty((B, TQ, CH), dtype=np.float32)
    for c in range(N_CORES):
        r = res.results[c]
        sl = slice(c * BPC, (c + 1) * BPC)
        attn[sl] = r["attnT"].astype(np.float32).transpose(0, 2, 1)
        out[sl] = r["outT"].astype(np.float32).transpose(0, 2, 1)
    return out, attn


# revision 29
# speedup vs baseline: 1.0190x; 1.0008x over previous
"""Trainium2 Bass kernel for the AttentionLayer problem.

Computation (per batch b):
    keys' = keys + sinenc(text_pos, w=1.385);  query' = query + sinenc(frame_pos, w=1.0)
    q = query' @ Wq + bq ; k = keys' @ Wk + bk ; v = values @ Wv + bv
    scores = q @ k^T ; masked softmax over keys -> attn  (output 1)
    out = (attn @ v) * sqrt(1/512) @ Wo + bo             (output 2)

Device strategy: data-parallel over B=64 across 8 cores (8 batches/core).

Weight folding (the big lever): q and k appear ONLY in scores, and v/x
only flow into out, so adjacent projections fold into single GEMMs:
    scores = q'' @ (Wq Wk^T) @ k''^T   with q'' = query + pe_q, k'' = keys + pe_k
      -> device computes  k~ = Wqk @ k''^T  (k-proj shaped),
         then scores = k~^T-contracted with q'' directly: the entire
         q-projection disappears.  Per-query additive terms
         (q_row.bk, bq.bk) are softmax-invariant and dropped; the
         per-key term bq.(k_row+bk) is a host-computed scalar folded
         into the exp bias alongside the mask.
    out = attn @ (values @ (Wv Wo * sqrt(1/TK))) + (bv@Wo*scale + bo)
      -> Wvo := Wv@Wo*scale folds the v-projection and out-projection
         into one; attn rows sum to 1 so bv folds into the constant
         bias (as before).
PE work per batch drops 163->98 streamed 512-col matmul units (-40%).
Numerics (host sim vs f64 reference): out 2.5e-3 / attn 3.6e-3
rel-to-max -- slightly BETTER than the unfolded fp16 scheme (fewer
quantization points), vs the 2e-2 gate.

Transposed layout ([feature, time]) throughout; no on-device transposes:
    k~T[eq,k] = WqkT^T @ keysT + Kpe    (Kpe = ((pe_k@Wk+bk)@Wq^T)^T, host)
    v~T[k,c]  = valuesT^T-slices @ Wvo
    scoresT[k,q] = k~T^T @ q''T          exp via ACT(Exp, bias=mask+ck)
    denom[q] = ones^T @ (tree-summed exp)    attnT = expT * (1/denom)
    outT[c,q] = v~^T @ attnT + bo'       (single GEMM, bias in the drain)

Dtype: all matmuls fp16 (e5m10), f32 PSUM; exp/denominator in f32
(scores reach +62, exp ~1e27).  fp8 was measured (host sim) at 0.4-0.9
rel err on either path -- unusable.  attn and out are DMA'd as fp16 and
widened on host.

Pipelining: per-batch phases issue as
    v~proj(b) | k~proj(b) | sums(b-1) | scores+exp(b) | x+out(b-1)
so the exp->tree-sum chain of b-1 is covered by ~7us of projection
matmuls and the normalized attn tiles are ready before x(b-1) needs
them.  Warmup matmuls run on a memset tile (no DMA dependency) so the
PE p-state ramp is paid during the initial input DMAs.  The final
batch's out DMAs fan out across all four engine queues to shorten the
drain tail.
"""

import math
import os

import numpy as np

import concourse.tile as tile
from concourse import bacc, mybir
from concourse.bass_utils import run_bass_kernel_spmd

dt = mybir.dt
F32 = dt.float32
F32R = dt.float32r
FP16 = dt.float16
AF = mybir.ActivationFunctionType

B, TQ, TK = 64, 1024, 512
CH = 512          # conv_channels == embed_dim == att_hid
N_CORES = 8
BPC = B // N_CORES  # batches per core
KEY_POS_RATE = 1.385
QUERY_POS_RATE = 1.0
OUT_SCALE = math.sqrt(1.0 / TK)
MASK_NEG = -1.0e30
N_WARMUP = 64

_LAST_EXEC_NS = None
_LAST_RES = None


def _sin_pos_enc(pos, w, d):
    """Reference-exact sinusoidal table for one position vector. [T, d] f32."""
    pos = pos.astype(np.float32)
    i = np.arange(d)
    inv_freq = np.power(np.float32(10000.0), -(2.0 * (i // 2)).astype(np.float32) / d)
    ang = (pos * np.float32(w))[:, None] * inv_freq[None, :]
    pe = np.where(i[None, :] % 2 == 0, np.sin(ang), np.cos(ang)).astype(np.float32)
    pe[pos == 0] = 0.0
    return pe


def _build_program(n_batch, keff=TK):
    nc = bacc.Bacc("TRN2", target_bir_lowering=False, debug=False, num_devices=1)

    qT_d = nc.dram_tensor("qT", [n_batch, CH, TQ], FP16, kind="ExternalInput")
    kT_d = nc.dram_tensor("kT", [n_batch, CH, TK], FP16, kind="ExternalInput")
    vT_d = nc.dram_tensor("vT", [n_batch, CH, TK], FP16, kind="ExternalInput")
    kpe_d = nc.dram_tensor("kpe", [CH, TK], FP16, kind="ExternalInput")
    wqk_d = nc.dram_tensor("wqk", [CH, CH], FP16, kind="ExternalInput")
    wvo_d = nc.dram_tensor("wvo", [CH, CH], FP16, kind="ExternalInput")
    bo_d = nc.dram_tensor("bo", [128, CH], FP16, kind="ExternalInput")
    mb_d = nc.dram_tensor("mb", [n_batch, TK], F32, kind="ExternalInput")
    ones_d = nc.dram_tensor("ones", [128, 128], F32R, kind="ExternalInput")

    attn_d = nc.dram_tensor("attnT", [n_batch, TK, TQ], FP16, kind="ExternalOutput")
    out_d = nc.dram_tensor("outT", [n_batch, CH, TQ], FP16, kind="ExternalOutput")

    NQ2 = TQ // 512                   # 2 query chunks of 512
    NCT = CH // 128                   # 4 feature tiles
    NKT = TK // 128                   # 4 key tiles
    s512 = lambda c: slice(c * 512, (c + 1) * 512)
    s128 = lambda t: slice(t * 128, (t + 1) * 128)

    with tile.TileContext(nc) as tc:
        with (
            tc.tile_pool(name="wpool", bufs=1) as wpool,
            tc.tile_pool(name="qin", bufs=3) as p_qin,
            tc.tile_pool(name="kin", bufs=3) as p_kin,
            tc.tile_pool(name="vin", bufs=3) as p_vin,
            tc.tile_pool(name="kt", bufs=5) as p_kt,
            tc.tile_pool(name="vt", bufs=8) as p_vt,
            tc.tile_pool(name="vt7", bufs=4) as p_vt7,
            tc.tile_pool(name="exp", bufs=5) as p_exp,
            tc.tile_pool(name="esum", bufs=1) as p_esum,
            tc.tile_pool(name="rec", bufs=2) as p_rec,
            tc.tile_pool(name="attnb", bufs=3) as p_attnb,
            tc.tile_pool(name="outt", bufs=2) as p_out,
            tc.tile_pool(name="ps", bufs=8, space="PSUM") as p_ps,
        ):
            ps_one = lambda nm: p_ps.tile([128, 512], F32, name=nm, tag="ps")

            # ---- warmup: lift the PE p-state during initial DMAs.
            # memset (ScalarE) instead of a DMA'd table so the first
            # warmup matmul issues ~us after kernel start, before the
            # DMA rings have even initialized. ----
            warm = wpool.tile([128, 128], FP16, name="warm")
            nc.vector.memset(warm[:], 1.0)
            for i in range(N_WARMUP):
                ps = ps_one(f"warm{i}")
                nc.tensor.matmul(
                    ps[:, :128], warm[:], warm[:],
                    start=True, stop=True,
                )

            # ---- resident weights/constants ----
            # ONE coalesced descriptor per tensor: each dma_start costs
            # ~0.7-1us of sequencer issue time (DIRECT2D), and the serial
            # issue stream -- not DMA bandwidth -- paces the lead-in.
            # sync queue carries the critical path: wvo + batch-0 inputs.
            def load_w(name, dram, dtype, engine=nc.sync):
                t = wpool.tile([128, NCT, CH], dtype, name=name)
                engine.dma_start(
                    t[:], dram.ap().rearrange("(c p) e -> p c e", p=128)
                )
                return [t[:, ct, :] for ct in range(NCT)]

            wqk_w = load_w("wqk", wqk_d, FP16)

            # one-time tables on the gpsimd queue (sync stays free for
            # batch-0 inputs).  bo is pre-broadcast to [128, CH] on host
            # and folded into v~ at the vproj drain (attn rows sum to 1,
            # so adding bo to every v~ row is exact) -- the x drains
            # become plain copies.
            bo_sb = wpool.tile([128, CH], FP16, name="bob")
            nc.gpsimd.dma_start(bo_sb[:], bo_d.ap())
            # exp bias (mask + per-key score bias) for all batches in one
            # transfer: col b*NKT+j holds mb[b, j*128:(j+1)*128]
            mb_all = wpool.tile([128, n_batch * NKT], F32, name="mball")
            nc.gpsimd.dma_start(
                mb_all[:], mb_d.ap().rearrange("b (j p) -> p (b j)", p=128)
            )
            ones_sb = wpool.tile([128, 128], F32R, name="ones")
            nc.gpsimd.dma_start(ones_sb[:], ones_d.ap())
            kpe_sb = []  # loaded lazily after batch-0 inputs (first use
            # is the kproj drain, ~7us in; keeps the lead-in DMA window
            # free for the critical first-batch tiles)

            state = {}

            def load_in(pool, dram, b, width, name, engine):
                t = pool.tile([128, NCT, width], FP16, name=name, tag=name[:3])
                engine.dma_start(
                    t[:], dram.ap()[b].rearrange("(c p) x -> p c x", p=128)
                )
                return [t[:, ct, :] for ct in range(NCT)]

            # ALL loads on the sync DGE queue, issued in first-use
            # order: a single queue delivers strictly FIFO, so the
            # first-needed tensor completes after exactly its own bytes.
            # (Measured: splitting across two queues makes the rings
            # interleave both streams and the first tensor arrives at
            # the fair-share point -- 7us later.)
            def load_qin(b):
                return load_in(p_qin, qT_d, b, TQ, f"qin{b}", nc.sync)

            def load_kin(b):
                # only the first keff key columns are ever read (masked
                # tail never reaches the output)
                t = p_kin.tile([128, NCT, keff], FP16, name=f"kin{b}", tag="kin")
                nc.sync.dma_start(
                    t[:],
                    kT_d.ap()[b, :, :keff].rearrange("(c p) x -> p c x", p=128),
                )
                return [t[:, ct, :] for ct in range(NCT)]

            def load_vin(b):
                return load_in(p_vin, vT_d, b, TK, f"vin{b}", nc.sync)

            def kv_part(b, last=False):
                # ---- inputs, in first-use order (kproj leads: its
                # weights+inputs are the first 1MB on the wire, so the
                # PE gets ~3.5us of kproj cover while vin streams) ----
                kin = load_kin(b)
                if state.get("wvo") is None:
                    state["wvo"] = load_w("wvo", wvo_d, FP16)
                vin = load_vin(b)
                qin = load_qin(b)
                if not kpe_sb:
                    t = wpool.tile([128, NCT, TK], FP16, name="kpe")
                    nc.gpsimd.dma_start(
                        t[:], kpe_d.ap().rearrange("(c p) k -> p c k", p=128)
                    )
                    kpe_sb.extend(t[:, ct, :] for ct in range(NCT))
                wvo = state["wvo"]

                # ---- k~ = Wqk @ keys''T (+ table): [eq, k] tiles ----
                # masked trailing keys (cols keff:TK) never reach the
                # output: their exp is forced to exact 0 by the -1e30
                # bias, so k~ for them may hold stale data -- stream only
                # the first keff columns
                kt = []
                for ht in range(NCT):
                    ps = ps_one(f"psk{b}_{ht}")
                    for ct in range(NCT):
                        nc.tensor.matmul(
                            ps[:, :keff], wqk_w[ct][:, s128(ht)],
                            kin[ct][:, :keff],
                            start=(ct == 0), stop=(ct == NCT - 1),
                        )
                    t = p_kt.tile([128, TK], FP16, name=f"kt{b}_{ht}", tag="kt")
                    if keff < TK and b < 2:
                        # first touch of each of the 5 pool buffers: the
                        # stale-col region must be finite (NaN would
                        # poison exp despite the -1e30 bias)
                        nc.vector.memset(t[:, keff:], 0.0)
                    nc.vector.tensor_add(
                        t[:, :keff], ps[:, :keff], kpe_sb[ht][:, :keff]
                    )
                    kt.append(t)


                # ---- v~ = values @ (Wv Wo * scale) + bo: [k, c] tiles.
                # The last batch's v~ is kept f32r so its x-phase can
                # consume the unnormalized exp tiles directly. ----
                vt = []
                for ktile in range(NKT):
                    ps = ps_one(f"psv{b}_{ktile}")
                    for ct in range(NCT):
                        nc.tensor.matmul(
                            ps[:], vin[ct][:, s128(ktile)], wvo[ct][:],
                            start=(ct == 0), stop=(ct == NCT - 1),
                        )
                    if last:
                        t = p_vt7.tile([128, CH], F32R, name=f"vt7_{ktile}",
                                       tag="vt7")
                    else:
                        t = p_vt.tile([128, CH], FP16, name=f"vt{b}_{ktile}",
                                      tag="vt")
                    nc.vector.tensor_add(t[:], ps[:], bo_sb[:])
                    vt.append(t)
                return qin, kt, vt

            def scores_exp(b, qt, kt):
                # ---- scores + exp (mask & key-bias folded into bias) ----
                expt = []
                for ktile in range(NKT):
                    ps = [ps_one(f"pss{b}_{ktile}_{c}") for c in range(NQ2)]
                    for ht in range(NCT):
                        for c in range(NQ2):
                            nc.tensor.matmul(
                                ps[c][:], kt[ht][:, s128(ktile)],
                                qt[ht][:, s512(c)],
                                start=(ht == 0), stop=(ht == NCT - 1),
                            )
                    t = p_exp.tile([128, TQ], F32R, name=f"exp{b}_{ktile}", tag="exp")
                    for c in range(NQ2):
                        nc.scalar.activation(
                            t[:, s512(c)], ps[c][:], AF.Exp,
                            bias=mb_all[:, b * NKT + ktile:b * NKT + ktile + 1],
                        )
                    expt.append(t)
                return expt

            def sums_recip(b, expt):
                # tree-sum the 4 exp tiles on GpSimd, then 2 ones-matmuls
                s01 = p_esum.tile([128, TQ], F32R, name=f"s01_{b}", tag="esA")
                s23 = p_esum.tile([128, TQ], F32R, name=f"s23_{b}", tag="esB")
                s = p_esum.tile([128, TQ], F32R, name=f"s_{b}", tag="esC")
                nc.gpsimd.tensor_add(s01[:], expt[0][:], expt[1][:])
                nc.gpsimd.tensor_add(s23[:], expt[2][:], expt[3][:])
                nc.gpsimd.tensor_add(s[:], s01[:], s23[:])
                rec = p_rec.tile([128, TQ], F32, name=f"rec{b}", tag="rec")
                for c in range(NQ2):
                    ps = ps_one(f"pssum{b}_{c}")
                    nc.tensor.matmul(
                        ps[:], ones_sb[:], s[:, s512(c)],
                        start=True, stop=True,
                    )
                    nc.vector.reciprocal_approx_fast(rec[:, s512(c)], ps[:])
                return rec

            def attn_norm(b, expt, rec):
                # one big [128, NKT*TQ] tile, one output descriptor
                t = p_attnb.tile(
                    [128, NKT, TQ], FP16, name=f"ab{b}", tag="attnb"
                )
                attn = [t[:, ktile, :] for ktile in range(NKT)]
                for ktile in range(NKT):
                    for c in range(NQ2):
                        nc.vector.tensor_mul(
                            attn[ktile][:, s512(c)], expt[ktile][:, s512(c)],
                            rec[:, s512(c)],
                        )
                nc.gpsimd.dma_start(
                    attn_d.ap()[b].rearrange("(j p) q -> p j q", p=128), t[:]
                )
                return attn

            def x_out(b, vt, attn):
                # out = attn @ (v~ + bo) in one GEMM; drain is a plain
                # copy into one big tile, one output descriptor
                t = p_out.tile([128, NCT, TQ], FP16, name=f"ot{b}", tag="outt")
                for ct in range(NCT):
                    ps = [ps_one(f"psx{b}_{ct}_{c}") for c in range(NQ2)]
                    for ktile in range(NKT):
                        for c in range(NQ2):
                            nc.tensor.matmul(
                                ps[c][:], vt[ktile][:, s128(ct)],
                                attn[ktile][:, s512(c)],
                                start=(ktile == 0), stop=(ktile == NKT - 1),
                            )
                    for c in range(NQ2):
                        # alternate psum-drain engines so the 8 drains don't
                        # serialize on one engine
                        dst = t[:, ct, s512(c)]
                        if ct % 2 == 0:
                            nc.scalar.copy(dst, ps[c][:])
                        else:
                            nc.vector.tensor_copy(dst, ps[c][:])
                nc.gpsimd.dma_start(
                    out_d.ap()[b].rearrange("(c p) q -> p c q", p=128), t[:]
                )

            def x_out_last(b, vt, expt, rec):
                # Final batch: consume UNNORMALIZED exp tiles (f32r) and
                # fold 1/denom into the drain multiply, so the x matmuls
                # start right after the sums matmuls with no wait on
                # attn-norm.  rec columns are partition-replicated, so the
                # drain is a single tensor-tensor multiply per chunk.
                # attn-norm muls interleave here, split Vector/GpSimd, so
                # no single engine serializes the endgame; attn DMAs flush
                # per-tile during the x matmuls.
                at = p_attnb.tile(
                    [128, NKT, TQ], FP16, name=f"ab{b}", tag="attnb"
                )
                dma_engines = [nc.gpsimd, nc.scalar, nc.sync]
                for ct in range(NCT):
                    ps = [ps_one(f"psx{b}_{ct}_{c}") for c in range(NQ2)]
                    for ktile in range(NKT):
                        for c in range(NQ2):
                            nc.tensor.matmul(
                                ps[c][:], vt[ktile][:, s128(ct)],
                                expt[ktile][:, s512(c)],
                                start=(ktile == 0), stop=(ktile == NKT - 1),
                            )
                    # attn tile `ct`: vector does chunk 0, gpsimd chunk 1
                    asl = at[:, ct, :]
                    nc.vector.tensor_mul(
                        asl[:, s512(0)], expt[ct][:, s512(0)], rec[:, s512(0)]
                    )
                    nc.gpsimd.tensor_mul(
                        asl[:, s512(1)], expt[ct][:, s512(1)], rec[:, s512(1)]
                    )
                    nc.gpsimd.dma_start(attn_d.ap()[b, s128(ct), :], asl)
                    t = p_out.tile([128, TQ], FP16, name=f"ot7_{ct}", tag="out7")
                    for c in range(NQ2):
                        nc.vector.tensor_mul(
                            t[:, s512(c)], ps[c][:], rec[:, s512(c)]
                        )
                        # chunked final store across three engine queues:
                        # start draining sooner, flush faster
                        eng = dma_engines[(ct * NQ2 + c) % 3]
                        eng.dma_start(
                            out_d.ap()[b, s128(ct), s512(c)], t[:, s512(c)]
                        )

            # 2-deep pipeline: per iteration
            #   v~proj(b) k~proj(b) | sums(b-1) | scores+exp(b) | x+out(b-1)
            # exp(b-1)'s tree-sum is covered by both projections (~7us);
            # recip+attn-norm(b-1) by scores(b).
            prev = None  # (expt, vt) of previous batch
            for b in range(n_batch):
                qin, kt, vt = kv_part(b, last=(b == n_batch - 1))
                if prev is not None:
                    rec = sums_recip(b - 1, prev[0])
                    attn = attn_norm(b - 1, prev[0], rec)
                expt = scores_exp(b, qin, kt)
                if prev is not None:
                    x_out(b - 1, prev[1], attn)
                prev = (expt, vt)
            b = n_batch - 1
            rec = sums_recip(b, prev[0])
            x_out_last(b, prev[1], prev[0], rec)
    nc.compile()
    return nc


def _host_prep(inputs):
    query = np.asarray(inputs["query"], dtype=np.float32)
    keys = np.asarray(inputs["keys"], dtype=np.float32)
    values = np.asarray(inputs["values"], dtype=np.float32)
    tpos = np.asarray(inputs["text_positions"])
    fpos = np.asarray(inputs["frame_positions"])
    mask = np.asarray(inputs["mask"])
    Wq = np.asarray(inputs["Wq"], dtype=np.float32)
    Wk = np.asarray(inputs["Wk"], dtype=np.float32)
    Wv = np.asarray(inputs["Wv"], dtype=np.float32)
    Wo = np.asarray(inputs["Wo"], dtype=np.float32)
    bq = np.asarray(inputs["bq"], dtype=np.float32)
    bk = np.asarray(inputs["bk"], dtype=np.float32)
    bv = np.asarray(inputs["bv"], dtype=np.float32)
    bo = np.asarray(inputs["bo"], dtype=np.float32)

    fshared = bool(np.all(fpos == fpos[0:1]))
    tshared = bool(np.all(tpos == tpos[0:1]))

    # q'' = query + pe_q, quantized once (no q-projection on device)
    if fshared:
        peq = _sin_pos_enc(fpos[0], QUERY_POS_RATE, CH)       # [TQ, CH]
        qT = (query + peq[None]).transpose(0, 2, 1)
    else:
        peq = np.stack([_sin_pos_enc(p, QUERY_POS_RATE, CH) for p in fpos])
        qT = (query + peq).transpose(0, 2, 1)

    # k'' = keys + pe_k; pe routed through the f32 table when shared
    wkq = Wk @ Wq.T                                            # [ek, eq]
    if tshared:
        pek = _sin_pos_enc(tpos[0], KEY_POS_RATE, CH)          # [TK, CH]
        kT = keys.transpose(0, 2, 1)
        kpe = np.ascontiguousarray(((pek @ Wk + bk) @ Wq.T).T)  # [eq, TK]
        keys_full = keys + pek[None]
    else:
        pek = np.stack([_sin_pos_enc(p, KEY_POS_RATE, CH) for p in tpos])
        kT = (keys + pek).transpose(0, 2, 1)
        kpe = np.ascontiguousarray(
            np.broadcast_to((bk @ Wq.T)[:, None], (CH, TK))
        ).astype(np.float32)
        keys_full = keys + pek

    qT = np.ascontiguousarray(qT).astype(np.float16)
    kT = np.ascontiguousarray(kT).astype(np.float16)
    vT = np.ascontiguousarray(values.transpose(0, 2, 1)).astype(np.float16)

    # exp bias: mask plus the per-key bq.(k_row+bk) term the q-side
    # folding moved out of the scores GEMM (zero when bq == 0)
    ck = keys_full @ (Wk @ bq) + np.float32(bk @ bq)           # [B, TK]
    mb = np.where(mask, np.float32(MASK_NEG), np.float32(0.0)) + ck
    mb = mb.astype(np.float32)
    ones = np.ones((128, 128), dtype=np.float32)

    # masked-tail fast path: if every batch masks exactly the last 64
    # keys, the k-projection can skip those columns (their exp is forced
    # to exact zero by the bias regardless of the k~ values there)
    keff = TK
    if bool(np.array_equal(
        mask, np.broadcast_to(np.arange(TK) >= TK - 64, mask.shape)
    )):
        keff = TK - 64

    bo2 = (np.float32(OUT_SCALE) * (bv @ Wo) + bo).astype(np.float32)
    shared = {
        "wqk": wkq.astype(np.float16),
        "wvo": (Wv @ Wo * np.float32(OUT_SCALE)).astype(np.float16),
        "kpe": kpe.astype(np.float16),
        "bo": np.ascontiguousarray(
            np.broadcast_to(bo2[None, :], (128, CH))
        ).astype(np.float16),
        "ones": ones,
    }
    in_maps = []
    for c in range(N_CORES):
        sl = slice(c * BPC, (c + 1) * BPC)
        m = dict(shared)
        m["qT"] = qT[sl]
        m["kT"] = kT[sl]
        m["vT"] = vT[sl]
        m["mb"] = mb[sl]
        in_maps.append(m)
    return in_maps, keff


def kernel(**inputs):
    global _LAST_EXEC_NS, _LAST_RES
    in_maps, keff = _host_prep(inputs)
    nc = _build_program(BPC, keff)
    trace = bool(int(os.environ.get("KERNEL_PROFILE", "0")))
    res = run_bass_kernel_spmd(nc, in_maps, list(range(N_CORES)), trace=trace)
    _LAST_EXEC_NS = res.exec_time_ns
    _LAST_RES = res

    attn = np.empty((B, TQ, TK), dtype=np.float32)
    out = np.empty((B, TQ, CH), dtype=np.float32)
    for c in range(N_CORES):
        r = res.results[c]
        sl = slice(c * BPC, (c + 1) * BPC)
        attn[sl] = r["attnT"].astype(np.float32).transpose(0, 2, 1)
        out[sl] = r["outT"].astype(np.float32).transpose(0, 2, 1)
    return out, attn
